# revision 46
# baseline (speedup 1.0000x reference)
"""Multi-head self-attention kernel for Trainium2 (8 NeuronCores).

Problem: B=2, S=2048, D=1024, H=16 heads of hd=64.
Sharding: core c handles batch b=c//4 and head-group hg=c%4 (4 heads each).

Per-core plan (all matmuls bf16, 1 cycle/row):
  qk^T = W_qk @ x^T          [512, 2048]   (q,k for 4 local heads, +bias)
  v    = x @ W_v^T           [2048, 256]   (natural layout, +bias, +ones col,
                                            emitted in per-head-pair halves)
  s^T[kj,qi] = k_h^T q_h     per head, per 512-wide qi chunk (K=64)
  e^T = exp(s^T / 8)  bf16   (no max subtraction: |s/8| <~ 2, safe)
  ctx[qi,d]  = sum_kj e^T[kj,qi]^T v[kj,d]   NATURAL layout: lhsT = e^T block
               [kj128, qi128], rhs = v [kj128, 65] -> 65-wide moving operand =
               2x fewer PE cycles than the transposed form; col 64 = denom.
               One open PSUM accumulation group per head at a time (PSUM
               zero-regions are bank-wide), lagging the exp production by one
               (pair, qi-chunk) phase.
  normalize on DVE with per-partition scalar (1/denom), write [qi,128] tiles
               pairing the two heads of the group -> cols 0:64 | 64:128
  transpose  ctx [qi,128] -> ctx^T [d,qi] via DMA XBAR (off the PE)
  out^T = W_p_cols @ ctx^T   [1024, 2048]  row-parallel partial projection,
               bf16 partials DMA'd out; host sums 4 partials + b_proj.
"""

import sys

sys.path.insert(0, "/opt/trn_rl_repo")

import ml_dtypes
import numpy as np

import concourse.bass as bass
import concourse.tile as tile
from concourse import bacc, mybir
from concourse.bass_utils import run_bass_kernel_spmd

B, S, D = 2, 2048, 1024
H, HD = 16, 64
HL = 4  # heads per core
P = 128
KC = D // P  # 8 contraction chunks over D
NQ = 4  # qi chunks of 512
NKJ = 16  # kj chunks of 128
F32 = mybir.dt.float32
BF16 = mybir.dt.bfloat16
F8 = mybir.dt.float8e4


def build_program():
    nc = bacc.Bacc("TRN2", target_bir_lowering=False)

    xt_d = nc.dram_tensor("xt", [D, S], BF16, kind="ExternalInput")
    wqk_d = nc.dram_tensor("wqk", [D, 2 * HL * HD], BF16, kind="ExternalInput")
    bqk_d = nc.dram_tensor("bqk", [2 * HL * HD], F32, kind="ExternalInput")
    wv_d = nc.dram_tensor("wv", [D, HL * HD], BF16, kind="ExternalInput")
    bv_d = nc.dram_tensor("bv", [HL * HD], F32, kind="ExternalInput")
    wp_d = nc.dram_tensor("wp", [HL * HD, D], BF16, kind="ExternalInput")
    out_d = nc.dram_tensor("out", [D, S], BF16, kind="ExternalOutput")

    out_v = out_d.rearrange("(mo p) s -> p mo s", p=P)  # [128, 8, 2048]

    with tile.TileContext(nc) as tc:
        with (
            tc.tile_pool(name="const", bufs=1) as const,
            tc.tile_pool(name="xp", bufs=1) as xp,
            tc.tile_pool(name="pexp", bufs=1) as pexp,
            tc.tile_pool(name="prc", bufs=1) as prc,
            tc.tile_pool(name="pcn", bufs=1) as pcn,
            tc.tile_pool(name="pout", bufs=1) as pout,
            tc.tile_pool(name="ps_mm", bufs=2, space="PSUM") as ps_mm,
            tc.tile_pool(name="ps_s", bufs=1, space="PSUM") as ps_s,
            tc.tile_pool(name="ps_o", bufs=1, space="PSUM") as ps_o,
        ):
            # dummy exp so the ACT table load happens during the input DMAs,
            # not on the first real exp
            dum = const.tile([1, 2], F32)
            nc.vector.memset(dum[:], 0.0)
            nc.scalar.activation(dum[:], dum[:], mybir.ActivationFunctionType.Exp)

            # ---- input DMAs, ordered by first use: k-half of wqk, x n0,
            # q-half, x n1, v weights, rest ----
            wqk_v = wqk_d.rearrange("(kc p) m -> p kc m", p=P)
            wqk_k = const.tile([P, KC, 256], BF16, tag="wqk_k")
            nc.sync.dma_start(wqk_k[:], wqk_v[:, :, 256:512])
            bqk_sb = const.tile([P, 4], F32)
            nc.sync.dma_start(bqk_sb[:], bqk_d.rearrange("(m p) -> p m", p=P))
            wv_sb = const.tile([P, KC, 256], BF16)
            bvb_sb = const.tile([P, 256], F32)
            xt_sb = [
                xp.tile([P, KC, 512], BF16, tag=f"xt{n}", name=f"xt{n}")
                for n in range(NQ)
            ]
            xt_v = xt_d.rearrange("(kc p) s -> p kc s", p=P)
            nc.sync.dma_start(xt_sb[0][:], xt_v[:, :, 0:512])
            wqk_q = const.tile([P, KC, 256], BF16, tag="wqk_q")
            nc.sync.dma_start(wqk_q[:], wqk_v[:, :, 0:256])
            nc.sync.dma_start(xt_sb[1][:], xt_v[:, :, 512:1024])
            nc.sync.dma_start(wv_sb[:], wv_d.rearrange("(kc p) m -> p kc m", p=P))
            nc.sync.dma_start(bvb_sb[:], bv_d[:].unsqueeze(0).broadcast_to([P, 256]))
            # xt2/xt3/wp are deferred into the fill schedule so the early q/k
            # remap DMAs aren't queued behind them on the DMA engines
            wp_sb = const.tile([P, 2, D], BF16)

            # ---- qk projection: qk^T [512, 2048], m-chunks 0,1 = q / 2,3 = k.
            # q/k live as fp8e4m3 in two layouts: the natural [128, 512]
            # bias-add output, and the DoubleRow remap [32, (head, khalf),
            # 512] produced by partition-moving DMAs ----
            qk_sb = [
                [
                    const.tile([P, 512], BF16, tag=f"qk{m}n{n}", name=f"qk{m}n{n}")
                    for n in range(NQ)
                ]
                for m in range(4)
            ]
            v_sb = [
                const.tile([P, HL * 65], BF16, tag=f"v{s}", name=f"v{s}")
                for s in range(NKJ)
            ]

            def emit_qk_chunk(m, n):
                pst = ps_mm.tile([P, 512], F32, tag="mm", name="pst")
                w = wqk_q if m < 2 else wqk_k
                mc = m % 2
                for kc in range(KC):
                    nc.tensor.matmul(
                        pst[:],
                        w[:, kc, mc * P : (mc + 1) * P],
                        xt_sb[n][:, kc, :],
                        start=(kc == 0),
                        stop=(kc == KC - 1),
                    )
                nc.vector.tensor_scalar_add(
                    qk_sb[m][n][:], pst[:], bqk_sb[:, m : m + 1]
                )

            def emit_v_chunk(s, half):
                # one head-pair (128 wide) of the v projection for kj chunk s
                pst = ps_mm.tile([P, 256], F32, tag="mm", name="pst")
                nsl = slice(half * P, (half + 1) * P)
                for kc in range(KC):
                    nc.tensor.matmul(
                        pst[:, 0:P],
                        xt_sb[s // 4][:, kc, (s % 4) * P : (s % 4 + 1) * P],
                        wv_sb[:, kc, nsl],
                        start=(kc == 0),
                        stop=(kc == KC - 1),
                    )
                vslice = v_sb[s][:].rearrange("p (h c) -> p h c", h=HL)[
                    :, 2 * half : 2 * half + 2, :
                ]
                psl = pst[:, 0:P].rearrange("p (h c) -> p h c", h=2)
                bsl = bvb_sb[:, nsl].rearrange("p (h c) -> p h c", h=2)
                nc.vector.tensor_add(vslice[:, :, 0:64], psl, bsl)
                # ones column (written as in*0+1 so it exists in bf16)
                nc.vector.tensor_scalar(
                    vslice[:, :, 64:65],
                    psl[:, :, 0:1],
                    0.0,
                    1.0,
                    mybir.AluOpType.mult,
                    mybir.AluOpType.add,
                )

            # ---- attention: 8 phases (pair pr, qi-chunk qc), pr alternating.
            # Scores+exp of phase p run while the AV of phase p-1 contracts
            # its 16 kj chunks one (head, qi-sub-block) accumulation group at
            # a time. ----
            ctxT = [
                [
                    [
                        const.tile(
                            [P, P], BF16, tag=f"ct{qc}p{c}s{sb}", name=f"ct{qc}p{c}s{sb}"
                        )
                        for sb in range(4)
                    ]
                    for c in range(2)
                ]
                for qc in range(NQ)
            ]

            ex_store = {}  # (pr, qc) -> list of (exA, exB) per g
            cn_cur = {}  # sb -> cn tile of the phase being reduced

            def emit_scores_exp(pr, qc, g):
                # fp8e4m3 DoubleRow scores: lhsT = k [32, 2, 128], rhs =
                # q [32, 2, 512] -> one instr per (kj chunk, head) at half
                # the bf16 row cost
                q_tile = qk_sb[pr][qc]
                # shared 3-deep ring (6 PSUM banks): lets the PE run ~1.5 g
                # ahead of ACT so exp never waits at phase boundaries
                psA = ps_s.tile([P, 1024], F32, tag="sA", bufs=1, name="psA")
                psB = ps_s.tile([P, 1024], F32, tag="sB", bufs=1, name="psB")
                for j in range(2):
                    kj = g * 2 + j
                    k_ap = qk_sb[2 + pr][kj // 4]
                    ksl = slice((kj % 4) * P, (kj % 4 + 1) * P)
                    nc.tensor.matmul(
                        psA[:, j * 512 : (j + 1) * 512],
                        k_ap[0:64, ksl],
                        q_tile[0:64, :],
                        start=True,
                        stop=True,
                    )
                exA = pexp.tile([P, 1024], BF16, tag="ex", bufs=34, name="exA")
                nc.scalar.activation(
                    exA[:], psA[:], mybir.ActivationFunctionType.Exp, scale=0.125
                )
                for j in range(2):
                    kj = g * 2 + j
                    k_ap = qk_sb[2 + pr][kj // 4]
                    ksl = slice((kj % 4) * P, (kj % 4 + 1) * P)
                    nc.tensor.matmul(
                        psB[:, j * 512 : (j + 1) * 512],
                        k_ap[64:128, ksl],
                        q_tile[64:128, :],
                        start=True,
                        stop=True,
                    )
                exB = pexp.tile([P, 1024], BF16, tag="ex", bufs=34, name="exB")
                nc.scalar.activation(
                    exB[:], psB[:], mybir.ActivationFunctionType.Exp, scale=0.125
                )
                ex_store[(pr, qc)].append((exA, exB))

            def emit_av_group(pr, qc, gidx, tail=False):
                # one (head, qi-sub-block) accumulation group: contract all 16
                # kj chunks of phase (pr, qc), then normalize; after the head-B
                # half of a sub-block, XBAR-transpose the [qi,128] ctx tile
                head, sb = gidx % 2, gidx // 2
                h = 2 * pr + head
                exs = ex_store[(pr, qc)]
                tag = "poA" if head == 0 else "poB"
                po = ps_o.tile([P, 65], F32, tag=tag, bufs=1, name=tag)
                for kj in range(NKJ):
                    ex = exs[kj // 2][head]
                    nc.tensor.matmul(
                        po[:],
                        ex[:, (kj % 2) * 512 + sb * P : (kj % 2) * 512 + (sb + 1) * P],
                        v_sb[kj][:, h * 65 : h * 65 + 65],
                        start=(kj == 0),
                        stop=(kj == NKJ - 1),
                    )
                rc = prc.tile([P, 1], F32, tag="rc", bufs=4, name="rc")
                nc.vector.reciprocal(rc[:], po[:, 64:65])
                if head == 0:
                    cn_cur[sb] = pcn.tile([P, P], BF16, tag="cn", bufs=4, name="cn")
                cn = cn_cur[sb]
                nc.vector.tensor_scalar_mul(
                    cn[:, head * 64 : head * 64 + 64], po[:, 0:64], rc[:]
                )
                if head == 1:
                    # tail: issue from the (then idle) ACT sequencer; the SP
                    # sequencer is backed up with out-DMAs there
                    eng = nc.scalar if tail else nc.sync
                    eng.dma_start_transpose(ctxT[qc][pr][sb][:], cn[:])

            def emit_proj_mo(qc, mo, tail=False):
                pp = ps_mm.tile([P, 512], F32, tag="mm", name="pp")
                for sb in range(4):
                    for kc2 in range(2):
                        nc.tensor.matmul(
                            pp[:, sb * P : (sb + 1) * P],
                            wp_sb[:, kc2, mo * P : (mo + 1) * P],
                            ctxT[qc][kc2][sb][:],
                            start=(kc2 == 0),
                            stop=(kc2 == 1),
                        )
                ot = pout.tile([P, 512], BF16, tag="ot", bufs=4, name="ot")
                nc.vector.tensor_copy(ot[:], pp[:])
                eng = nc.scalar if tail else nc.sync
                eng.dma_start(out_v[:, mo, qc * 512 : (qc + 1) * 512], ot[:])

            # Deferred bulk loads: a tiny copy into the destination tile that
            # reads an early-pipeline tile creates a WAW dependency, pinning
            # the DMA behind the q/k remaps in the queue (the scheduler hoists
            # dependency-free DMAs to t=0 otherwise)
            def dma_xt(n, dep):
                nc.vector.tensor_copy(
                    xt_sb[n][0:1, 0:1, 0:2], dep[0:1, 0:2].unsqueeze(1)
                )
                nc.sync.dma_start(xt_sb[n][:], xt_v[:, :, n * 512 : (n + 1) * 512])

            def dma_wp(dep):
                nc.vector.tensor_copy(
                    wp_sb[0:1, 0:1, 0:2], dep[0:1, 0:2].unsqueeze(1)
                )
                nc.sync.dma_start(wp_sb[:], wp_d.rearrange("(kc p) m -> p kc m", p=P))

            # fill work (qkv chunks) per (phase, g), emitted after that g's
            # scores so ACT never waits behind fills
            fills = {
                (0, 0): [lambda: dma_xt(2, qk_sb[2][0]), lambda: emit_qk_chunk(2, 1),
                         lambda: dma_xt(3, qk_sb[2][1])],
                (0, 1): [lambda: emit_v_chunk(0, 0), lambda: emit_v_chunk(1, 0),
                         lambda: emit_v_chunk(2, 0), lambda: emit_v_chunk(3, 0)],
                (0, 2): [lambda: emit_qk_chunk(2, 2), lambda: emit_v_chunk(4, 0),
                         lambda: emit_v_chunk(5, 0)],
                (0, 3): [lambda: emit_v_chunk(6, 0), lambda: emit_v_chunk(7, 0),
                         lambda: emit_v_chunk(8, 0)],
                (0, 4): [lambda: dma_wp(qk_sb[2][2]), lambda: emit_qk_chunk(2, 3),
                         lambda: emit_v_chunk(9, 0), lambda: emit_v_chunk(10, 0)],
                (0, 5): [lambda: emit_v_chunk(11, 0), lambda: emit_v_chunk(12, 0),
                         lambda: emit_v_chunk(13, 0)],
                (0, 6): [lambda: emit_qk_chunk(3, 0), lambda: emit_v_chunk(14, 0),
                         lambda: emit_v_chunk(15, 0)],
                (0, 7): [lambda: emit_qk_chunk(1, 0)],
                (1, 0): [lambda: emit_v_chunk(0, 1), lambda: emit_v_chunk(1, 1)],
                (1, 1): [lambda: emit_qk_chunk(3, 1), lambda: emit_v_chunk(2, 1),
                         lambda: emit_v_chunk(3, 1)],
                (1, 2): [lambda: emit_v_chunk(4, 1), lambda: emit_v_chunk(5, 1)],
                (1, 3): [lambda: emit_qk_chunk(3, 2), lambda: emit_v_chunk(6, 1),
                         lambda: emit_v_chunk(7, 1)],
                (1, 4): [lambda: emit_v_chunk(8, 1), lambda: emit_v_chunk(9, 1)],
                (1, 5): [lambda: emit_qk_chunk(3, 3), lambda: emit_v_chunk(10, 1),
                         lambda: emit_v_chunk(11, 1)],
                (1, 6): [lambda: emit_qk_chunk(0, 1), lambda: emit_v_chunk(12, 1),
                         lambda: emit_v_chunk(13, 1)],
                (1, 7): [lambda: emit_v_chunk(14, 1), lambda: emit_v_chunk(15, 1)],
                (2, 1): [lambda: emit_qk_chunk(1, 1)],
                (2, 3): [lambda: emit_qk_chunk(0, 2)],
                (3, 3): [lambda: emit_qk_chunk(1, 2)],
                (4, 3): [lambda: emit_qk_chunk(0, 3)],
                (5, 3): [lambda: emit_qk_chunk(1, 3)],
            }

            emit_qk_chunk(2, 0)
            emit_qk_chunk(0, 0)

            phases = [(p % 2, p // 2) for p in range(8)]
            for p, (pr, qc) in enumerate(phases):
                ex_store[(pr, qc)] = []
                for g in range(8):
                    if p <= 1:
                        # during the ramp the scores wait on the q/k remap
                        # DMAs anyway; front-running the fills keeps the PE
                        # busy instead of blocking in-order behind them
                        for fill in fills.get((p, g), []):
                            fill()
                        emit_scores_exp(pr, qc, g)
                    else:
                        emit_scores_exp(pr, qc, g)
                        for fill in fills.get((p, g), []):
                            fill()
                    if p >= 1:
                        emit_av_group(*phases[p - 1], g)
                    if p >= 3 and p % 2 == 1 and g >= 1:
                        # proj(qc) once ctxT[qc] is complete (end of phase
                        # 2qc+2); shifted one g late so mo0 never blocks the
                        # in-order PE on the last ctxT transpose's DMA sem
                        emit_proj_mo(p // 2 - 1, g - 1)
                        if g == 7:
                            emit_proj_mo(p // 2 - 1, 7)
            # tail: head-A groups first (they only need the A exps, which land
            # one exp earlier), then B groups whose transposes gate proj(3)
            for gidx in (0, 2, 4, 6, 1, 3, 5, 7):
                emit_av_group(*phases[7], gidx, tail=True)
            for mo in range(8):
                emit_proj_mo(NQ - 1, mo, tail=True)

    nc.compile()
    return nc


_NC = None


def _get_program():
    global _NC
    if _NC is None:
        _NC = build_program()
    return _NC


def prepare_inputs(x, w_qkv, b_qkv, w_proj):
    """Build the 8 per-core input maps from full inputs."""
    in_maps = []
    for c in range(8):
        b, hg = c // 4, c % 4
        sl = slice(hg * 256, (hg + 1) * 256)
        w_q, w_k, w_v = w_qkv[0:D][sl], w_qkv[D : 2 * D][sl], w_qkv[2 * D :][sl]
        in_maps.append(
            {
                "xt": np.ascontiguousarray(x[b].T).astype(ml_dtypes.bfloat16),
                "wqk": np.ascontiguousarray(np.vstack([w_q, w_k]).T).astype(
                    ml_dtypes.bfloat16
                ),
                "bqk": np.concatenate([b_qkv[0:D][sl], b_qkv[D : 2 * D][sl]]),
                "wv": np.ascontiguousarray(w_v.T).astype(ml_dtypes.bfloat16),
                "bv": np.ascontiguousarray(b_qkv[2 * D :][sl]),
                "wp": np.ascontiguousarray(w_proj[:, sl].T).astype(ml_dtypes.bfloat16),
            }
        )
    return in_maps


def run(in_maps, **kwargs):
    nc = _get_program()
    last_err = None
    for _ in range(3):
        try:
            res = run_bass_kernel_spmd(nc, in_maps, core_ids=list(range(8)), **kwargs)
            # force device->host materialization inside the retry scope: lazy
            # jax outputs can surface transient device errors at first access
            res.results = [
                {k: np.array(v, dtype=np.float32) for k, v in r.items()}
                for r in res.results
            ]
            return res
        except Exception as e:  # transient NRT_EXEC_UNIT_UNRECOVERABLE etc.
            last_err = e
    raise last_err


def assemble(results, b_proj):
    out = np.empty((B, S, D), dtype=np.float32)
    for b in range(B):
        acc = results[4 * b]["out"].copy()
        for hg in range(1, 4):
            acc += results[4 * b + hg]["out"]
        out[b] = acc.T + b_proj
    return out


def kernel(x, w_qkv, b_qkv, w_proj, b_proj):
    x = np.asarray(x, dtype=np.float32)
    w_qkv = np.asarray(w_qkv, dtype=np.float32)
    b_qkv = np.asarray(b_qkv, dtype=np.float32)
    w_proj = np.asarray(w_proj, dtype=np.float32)
    b_proj = np.asarray(b_proj, dtype=np.float32)
    res = run(prepare_inputs(x, w_qkv, b_qkv, w_proj))
    return assemble(res.results, b_proj)


# revision 47
# speedup vs baseline: 1.0016x; 1.0016x over previous
"""Multi-head self-attention kernel for Trainium2 (8 NeuronCores).

Problem: B=2, S=2048, D=1024, H=16 heads of hd=64.
Sharding: core c handles batch b=c//4 and head-group hg=c%4 (4 heads each).

Per-core plan (all matmuls bf16, 1 cycle/row):
  qk^T = W_qk @ x^T          [512, 2048]   (q,k for 4 local heads, +bias)
  v    = x @ W_v^T           [2048, 256]   (natural layout, +bias, +ones col,
                                            emitted in per-head-pair halves)
  s^T[kj,qi] = k_h^T q_h     per head, per 512-wide qi chunk (K=64)
  e^T = exp(s^T / 8)  bf16   (no max subtraction: |s/8| <~ 2, safe)
  ctx[qi,d]  = sum_kj e^T[kj,qi]^T v[kj,d]   NATURAL layout: lhsT = e^T block
               [kj128, qi128], rhs = v [kj128, 65] -> 65-wide moving operand =
               2x fewer PE cycles than the transposed form; col 64 = denom.
               One open PSUM accumulation group per head at a time (PSUM
               zero-regions are bank-wide), lagging the exp production by one
               (pair, qi-chunk) phase.
  normalize on DVE with per-partition scalar (1/denom), write [qi,128] tiles
               pairing the two heads of the group -> cols 0:64 | 64:128
  transpose  ctx [qi,128] -> ctx^T [d,qi] via DMA XBAR (off the PE)
  out^T = W_p_cols @ ctx^T   [1024, 2048]  row-parallel partial projection,
               bf16 partials DMA'd out; host sums 4 partials + b_proj.
"""

import sys

sys.path.insert(0, "/opt/trn_rl_repo")

import ml_dtypes
import numpy as np

import concourse.bass as bass
import concourse.tile as tile
from concourse import bacc, mybir
from concourse.bass_utils import run_bass_kernel_spmd

B, S, D = 2, 2048, 1024
H, HD = 16, 64
HL = 4  # heads per core
P = 128
KC = D // P  # 8 contraction chunks over D
NQ = 4  # qi chunks of 512
NKJ = 16  # kj chunks of 128
F32 = mybir.dt.float32
BF16 = mybir.dt.bfloat16
F8 = mybir.dt.float8e4


def build_program():
    nc = bacc.Bacc("TRN2", target_bir_lowering=False)

    xt_d = nc.dram_tensor("xt", [D, S], BF16, kind="ExternalInput")
    wqk_d = nc.dram_tensor("wqk", [D, 2 * HL * HD], BF16, kind="ExternalInput")
    bqk_d = nc.dram_tensor("bqk", [2 * HL * HD], F32, kind="ExternalInput")
    wv_d = nc.dram_tensor("wv", [D, HL * HD], BF16, kind="ExternalInput")
    bv_d = nc.dram_tensor("bv", [HL * HD], F32, kind="ExternalInput")
    wp_d = nc.dram_tensor("wp", [HL * HD, D], BF16, kind="ExternalInput")
    out_d = nc.dram_tensor("out", [D, S], BF16, kind="ExternalOutput")

    out_v = out_d.rearrange("(mo p) s -> p mo s", p=P)  # [128, 8, 2048]

    with tile.TileContext(nc) as tc:
        with (
            tc.tile_pool(name="const", bufs=1) as const,
            tc.tile_pool(name="xp", bufs=1) as xp,
            tc.tile_pool(name="pexp", bufs=1) as pexp,
            tc.tile_pool(name="prc", bufs=1) as prc,
            tc.tile_pool(name="pcn", bufs=1) as pcn,
            tc.tile_pool(name="pout", bufs=1) as pout,
            tc.tile_pool(name="ps_mm", bufs=2, space="PSUM") as ps_mm,
            tc.tile_pool(name="ps_s", bufs=1, space="PSUM") as ps_s,
            tc.tile_pool(name="ps_o", bufs=1, space="PSUM") as ps_o,
        ):
            # dummy exp so the ACT table load happens during the input DMAs,
            # not on the first real exp
            dum = const.tile([1, 2], F32)
            nc.vector.memset(dum[:], 0.0)
            nc.scalar.activation(dum[:], dum[:], mybir.ActivationFunctionType.Exp)

            # ---- input DMAs, ordered by first use: k-half of wqk, x n0,
            # q-half, x n1, v weights, rest ----
            wqk_v = wqk_d.rearrange("(kc p) m -> p kc m", p=P)
            wqk_k = const.tile([P, KC, 256], BF16, tag="wqk_k")
            nc.sync.dma_start(wqk_k[:], wqk_v[:, :, 256:512])
            bqk_sb = const.tile([P, 4], F32)
            nc.sync.dma_start(bqk_sb[:], bqk_d.rearrange("(m p) -> p m", p=P))
            wv_sb = const.tile([P, KC, 256], BF16)
            bvb_sb = const.tile([P, 256], F32)
            xt_sb = [
                xp.tile([P, KC, 512], BF16, tag=f"xt{n}", name=f"xt{n}")
                for n in range(NQ)
            ]
            xt_v = xt_d.rearrange("(kc p) s -> p kc s", p=P)
            nc.sync.dma_start(xt_sb[0][:], xt_v[:, :, 0:512])
            wqk_q = const.tile([P, KC, 256], BF16, tag="wqk_q")
            nc.sync.dma_start(wqk_q[:], wqk_v[:, :, 0:256])
            nc.sync.dma_start(xt_sb[1][:], xt_v[:, :, 512:1024])
            nc.sync.dma_start(wv_sb[:], wv_d.rearrange("(kc p) m -> p kc m", p=P))
            nc.sync.dma_start(bvb_sb[:], bv_d[:].unsqueeze(0).broadcast_to([P, 256]))
            # xt2/xt3/wp are deferred into the fill schedule so the early q/k
            # remap DMAs aren't queued behind them on the DMA engines
            wp_sb = const.tile([P, 2, D], BF16)

            # ---- qk projection: qk^T [512, 2048], m-chunks 0,1 = q / 2,3 = k.
            # q/k live as fp8e4m3 in two layouts: the natural [128, 512]
            # bias-add output, and the DoubleRow remap [32, (head, khalf),
            # 512] produced by partition-moving DMAs ----
            qk_sb = [
                [
                    const.tile([P, 512], BF16, tag=f"qk{m}n{n}", name=f"qk{m}n{n}")
                    for n in range(NQ)
                ]
                for m in range(4)
            ]
            v_sb = [
                const.tile([P, HL * 65], BF16, tag=f"v{s}", name=f"v{s}")
                for s in range(NKJ)
            ]

            def emit_qk_chunk(m, n):
                pst = ps_mm.tile([P, 512], F32, tag="mm", name="pst")
                w = wqk_q if m < 2 else wqk_k
                mc = m % 2
                for kc in range(KC):
                    nc.tensor.matmul(
                        pst[:],
                        w[:, kc, mc * P : (mc + 1) * P],
                        xt_sb[n][:, kc, :],
                        start=(kc == 0),
                        stop=(kc == KC - 1),
                    )
                nc.vector.tensor_scalar_add(
                    qk_sb[m][n][:], pst[:], bqk_sb[:, m : m + 1]
                )

            def emit_v_chunk(s, half):
                # one head-pair (128 wide) of the v projection for kj chunk s
                pst = ps_mm.tile([P, 256], F32, tag="mm", name="pst")
                nsl = slice(half * P, (half + 1) * P)
                for kc in range(KC):
                    nc.tensor.matmul(
                        pst[:, 0:P],
                        xt_sb[s // 4][:, kc, (s % 4) * P : (s % 4 + 1) * P],
                        wv_sb[:, kc, nsl],
                        start=(kc == 0),
                        stop=(kc == KC - 1),
                    )
                vslice = v_sb[s][:].rearrange("p (h c) -> p h c", h=HL)[
                    :, 2 * half : 2 * half + 2, :
                ]
                psl = pst[:, 0:P].rearrange("p (h c) -> p h c", h=2)
                bsl = bvb_sb[:, nsl].rearrange("p (h c) -> p h c", h=2)
                nc.vector.tensor_add(vslice[:, :, 0:64], psl, bsl)
                # ones column (written as in*0+1 so it exists in bf16)
                nc.vector.tensor_scalar(
                    vslice[:, :, 64:65],
                    psl[:, :, 0:1],
                    0.0,
                    1.0,
                    mybir.AluOpType.mult,
                    mybir.AluOpType.add,
                )

            # ---- attention: 8 phases (pair pr, qi-chunk qc), pr alternating.
            # Scores+exp of phase p run while the AV of phase p-1 contracts
            # its 16 kj chunks one (head, qi-sub-block) accumulation group at
            # a time. ----
            ctxT = [
                [
                    [
                        const.tile(
                            [P, P], BF16, tag=f"ct{qc}p{c}s{sb}", name=f"ct{qc}p{c}s{sb}"
                        )
                        for sb in range(4)
                    ]
                    for c in range(2)
                ]
                for qc in range(NQ)
            ]

            ex_store = {}  # (pr, qc) -> list of (exA, exB) per g
            cn_cur = {}  # sb -> cn tile of the phase being reduced

            def emit_scores_exp(pr, qc, g):
                # fp8e4m3 DoubleRow scores: lhsT = k [32, 2, 128], rhs =
                # q [32, 2, 512] -> one instr per (kj chunk, head) at half
                # the bf16 row cost
                q_tile = qk_sb[pr][qc]
                # shared 3-deep ring (6 PSUM banks): lets the PE run ~1.5 g
                # ahead of ACT so exp never waits at phase boundaries
                psA = ps_s.tile([P, 1024], F32, tag="sA", bufs=1, name="psA")
                psB = ps_s.tile([P, 1024], F32, tag="sB", bufs=1, name="psB")
                for j in range(2):
                    kj = g * 2 + j
                    k_ap = qk_sb[2 + pr][kj // 4]
                    ksl = slice((kj % 4) * P, (kj % 4 + 1) * P)
                    nc.tensor.matmul(
                        psA[:, j * 512 : (j + 1) * 512],
                        k_ap[0:64, ksl],
                        q_tile[0:64, :],
                        start=True,
                        stop=True,
                    )
                exA = pexp.tile([P, 1024], BF16, tag="ex", bufs=34, name="exA")
                nc.scalar.activation(
                    exA[:], psA[:], mybir.ActivationFunctionType.Exp, scale=0.125
                )
                for j in range(2):
                    kj = g * 2 + j
                    k_ap = qk_sb[2 + pr][kj // 4]
                    ksl = slice((kj % 4) * P, (kj % 4 + 1) * P)
                    nc.tensor.matmul(
                        psB[:, j * 512 : (j + 1) * 512],
                        k_ap[64:128, ksl],
                        q_tile[64:128, :],
                        start=True,
                        stop=True,
                    )
                exB = pexp.tile([P, 1024], BF16, tag="ex", bufs=34, name="exB")
                nc.scalar.activation(
                    exB[:], psB[:], mybir.ActivationFunctionType.Exp, scale=0.125
                )
                ex_store[(pr, qc)].append((exA, exB))

            def emit_av_group(pr, qc, gidx, tail=False):
                # one (head, qi-sub-block) accumulation group: contract all 16
                # kj chunks of phase (pr, qc), then normalize; after the head-B
                # half of a sub-block, XBAR-transpose the [qi,128] ctx tile
                head, sb = gidx % 2, gidx // 2
                h = 2 * pr + head
                exs = ex_store[(pr, qc)]
                tag = "poA" if head == 0 else "poB"
                po = ps_o.tile([P, 65], F32, tag=tag, bufs=1, name=tag)
                for kj in range(NKJ):
                    ex = exs[kj // 2][head]
                    nc.tensor.matmul(
                        po[:],
                        ex[:, (kj % 2) * 512 + sb * P : (kj % 2) * 512 + (sb + 1) * P],
                        v_sb[kj][:, h * 65 : h * 65 + 65],
                        start=(kj == 0),
                        stop=(kj == NKJ - 1),
                    )
                rc = prc.tile([P, 1], F32, tag="rc", bufs=4, name="rc")
                nc.vector.reciprocal(rc[:], po[:, 64:65])
                if head == 0:
                    cn_cur[sb] = pcn.tile([P, P], BF16, tag="cn", bufs=4, name="cn")
                cn = cn_cur[sb]
                nc.vector.tensor_scalar_mul(
                    cn[:, head * 64 : head * 64 + 64], po[:, 0:64], rc[:]
                )
                if head == 1:
                    # tail: issue from the (then idle) ACT sequencer; the SP
                    # sequencer is backed up with out-DMAs there
                    eng = nc.scalar if tail else nc.sync
                    eng.dma_start_transpose(ctxT[qc][pr][sb][:], cn[:])

            def emit_proj_mo(qc, mo, tail=False):
                pp = ps_mm.tile([P, 512], F32, tag="mm", name="pp")
                for sb in range(4):
                    for kc2 in range(2):
                        nc.tensor.matmul(
                            pp[:, sb * P : (sb + 1) * P],
                            wp_sb[:, kc2, mo * P : (mo + 1) * P],
                            ctxT[qc][kc2][sb][:],
                            start=(kc2 == 0),
                            stop=(kc2 == 1),
                        )
                ot = pout.tile([P, 512], BF16, tag="ot", bufs=4, name="ot")
                if tail and mo % 2 == 0:
                    # after the last exp the ACT engine is idle: split the
                    # final copy chain across ACT and DVE; out-DMAs stay on
                    # the SP sequencer so they don't serialize behind the
                    # ACT-issued transposes
                    nc.scalar.copy(ot[:], pp[:])
                else:
                    nc.vector.tensor_copy(ot[:], pp[:])
                nc.sync.dma_start(out_v[:, mo, qc * 512 : (qc + 1) * 512], ot[:])

            # Deferred bulk loads: a tiny copy into the destination tile that
            # reads an early-pipeline tile creates a WAW dependency, pinning
            # the DMA behind the q/k remaps in the queue (the scheduler hoists
            # dependency-free DMAs to t=0 otherwise)
            def dma_xt(n, dep):
                nc.vector.tensor_copy(
                    xt_sb[n][0:1, 0:1, 0:2], dep[0:1, 0:2].unsqueeze(1)
                )
                nc.sync.dma_start(xt_sb[n][:], xt_v[:, :, n * 512 : (n + 1) * 512])

            def dma_wp(dep):
                nc.vector.tensor_copy(
                    wp_sb[0:1, 0:1, 0:2], dep[0:1, 0:2].unsqueeze(1)
                )
                nc.sync.dma_start(wp_sb[:], wp_d.rearrange("(kc p) m -> p kc m", p=P))

            # fill work (qkv chunks) per (phase, g), emitted after that g's
            # scores so ACT never waits behind fills
            fills = {
                (0, 0): [lambda: dma_xt(2, qk_sb[2][0]), lambda: emit_qk_chunk(2, 1),
                         lambda: dma_xt(3, qk_sb[2][1])],
                (0, 1): [lambda: emit_v_chunk(0, 0), lambda: emit_v_chunk(1, 0),
                         lambda: emit_v_chunk(2, 0), lambda: emit_v_chunk(3, 0)],
                (0, 2): [lambda: emit_qk_chunk(2, 2), lambda: emit_v_chunk(4, 0),
                         lambda: emit_v_chunk(5, 0)],
                (0, 3): [lambda: emit_v_chunk(6, 0), lambda: emit_v_chunk(7, 0),
                         lambda: emit_v_chunk(8, 0)],
                (0, 4): [lambda: dma_wp(qk_sb[2][2]), lambda: emit_qk_chunk(2, 3),
                         lambda: emit_v_chunk(9, 0), lambda: emit_v_chunk(10, 0)],
                (0, 5): [lambda: emit_v_chunk(11, 0), lambda: emit_v_chunk(12, 0),
                         lambda: emit_v_chunk(13, 0)],
                (0, 6): [lambda: emit_qk_chunk(3, 0), lambda: emit_v_chunk(14, 0),
                         lambda: emit_v_chunk(15, 0)],
                (0, 7): [lambda: emit_qk_chunk(1, 0)],
                (1, 0): [lambda: emit_v_chunk(0, 1), lambda: emit_v_chunk(1, 1)],
                (1, 1): [lambda: emit_qk_chunk(3, 1), lambda: emit_v_chunk(2, 1),
                         lambda: emit_v_chunk(3, 1)],
                (1, 2): [lambda: emit_v_chunk(4, 1), lambda: emit_v_chunk(5, 1)],
                (1, 3): [lambda: emit_qk_chunk(3, 2), lambda: emit_v_chunk(6, 1),
                         lambda: emit_v_chunk(7, 1)],
                (1, 4): [lambda: emit_v_chunk(8, 1), lambda: emit_v_chunk(9, 1)],
                (1, 5): [lambda: emit_qk_chunk(3, 3), lambda: emit_v_chunk(10, 1),
                         lambda: emit_v_chunk(11, 1)],
                (1, 6): [lambda: emit_qk_chunk(0, 1), lambda: emit_v_chunk(12, 1),
                         lambda: emit_v_chunk(13, 1)],
                (1, 7): [lambda: emit_v_chunk(14, 1), lambda: emit_v_chunk(15, 1)],
                (2, 1): [lambda: emit_qk_chunk(1, 1)],
                (2, 3): [lambda: emit_qk_chunk(0, 2)],
                (3, 3): [lambda: emit_qk_chunk(1, 2)],
                (4, 3): [lambda: emit_qk_chunk(0, 3)],
                (5, 3): [lambda: emit_qk_chunk(1, 3)],
            }

            emit_qk_chunk(2, 0)
            emit_qk_chunk(0, 0)

            phases = [(p % 2, p // 2) for p in range(8)]
            for p, (pr, qc) in enumerate(phases):
                ex_store[(pr, qc)] = []
                for g in range(8):
                    if p <= 1:
                        # during the ramp the scores wait on the q/k remap
                        # DMAs anyway; front-running the fills keeps the PE
                        # busy instead of blocking in-order behind them
                        for fill in fills.get((p, g), []):
                            fill()
                        emit_scores_exp(pr, qc, g)
                    else:
                        emit_scores_exp(pr, qc, g)
                        for fill in fills.get((p, g), []):
                            fill()
                    if p >= 1:
                        emit_av_group(*phases[p - 1], g)
                    if p >= 3 and p % 2 == 1 and g >= 1:
                        # proj(qc) once ctxT[qc] is complete (end of phase
                        # 2qc+2); shifted one g late so mo0 never blocks the
                        # in-order PE on the last ctxT transpose's DMA sem
                        emit_proj_mo(p // 2 - 1, g - 1)
                        if g == 7:
                            emit_proj_mo(p // 2 - 1, 7)
            # tail: head-A groups first (they only need the A exps, which land
            # one exp earlier), then B groups whose transposes gate proj(3)
            for gidx in (0, 2, 4, 6, 1, 3, 5, 7):
                emit_av_group(*phases[7], gidx, tail=True)
            for mo in range(8):
                emit_proj_mo(NQ - 1, mo, tail=True)

    nc.compile()
    return nc


_NC = None


def _get_program():
    global _NC
    if _NC is None:
        _NC = build_program()
    return _NC


def prepare_inputs(x, w_qkv, b_qkv, w_proj):
    """Build the 8 per-core input maps from full inputs."""
    in_maps = []
    for c in range(8):
        b, hg = c // 4, c % 4
        sl = slice(hg * 256, (hg + 1) * 256)
        w_q, w_k, w_v = w_qkv[0:D][sl], w_qkv[D : 2 * D][sl], w_qkv[2 * D :][sl]
        in_maps.append(
            {
                "xt": np.ascontiguousarray(x[b].T).astype(ml_dtypes.bfloat16),
                "wqk": np.ascontiguousarray(np.vstack([w_q, w_k]).T).astype(
                    ml_dtypes.bfloat16
                ),
                "bqk": np.concatenate([b_qkv[0:D][sl], b_qkv[D : 2 * D][sl]]),
                "wv": np.ascontiguousarray(w_v.T).astype(ml_dtypes.bfloat16),
                "bv": np.ascontiguousarray(b_qkv[2 * D :][sl]),
                "wp": np.ascontiguousarray(w_proj[:, sl].T).astype(ml_dtypes.bfloat16),
            }
        )
    return in_maps


def run(in_maps, **kwargs):
    nc = _get_program()
    last_err = None
    for _ in range(3):
        try:
            res = run_bass_kernel_spmd(nc, in_maps, core_ids=list(range(8)), **kwargs)
            # force device->host materialization inside the retry scope: lazy
            # jax outputs can surface transient device errors at first access
            res.results = [
                {k: np.array(v, dtype=np.float32) for k, v in r.items()}
                for r in res.results
            ]
            return res
        except Exception as e:  # transient NRT_EXEC_UNIT_UNRECOVERABLE etc.
            last_err = e
    raise last_err


def assemble(results, b_proj):
    out = np.empty((B, S, D), dtype=np.float32)
    for b in range(B):
        acc = results[4 * b]["out"].copy()
        for hg in range(1, 4):
            acc += results[4 * b + hg]["out"]
        out[b] = acc.T + b_proj
    return out


def kernel(x, w_qkv, b_qkv, w_proj, b_proj):
    x = np.asarray(x, dtype=np.float32)
    w_qkv = np.asarray(w_qkv, dtype=np.float32)
    b_qkv = np.asarray(b_qkv, dtype=np.float32)
    w_proj = np.asarray(w_proj, dtype=np.float32)
    b_proj = np.asarray(b_proj, dtype=np.float32)
    res = run(prepare_inputs(x, w_qkv, b_qkv, w_proj))
    return assemble(res.results, b_proj)


# revision 48
# speedup vs baseline: 1.0064x; 1.0048x over previous
"""Multi-head self-attention kernel for Trainium2 (8 NeuronCores).

Problem: B=2, S=2048, D=1024, H=16 heads of hd=64.
Sharding: core c handles batch b=c//4 and head-group hg=c%4 (4 heads each).

Per-core plan (all matmuls bf16, 1 cycle/row):
  qk^T = W_qk @ x^T          [512, 2048]   (q,k for 4 local heads, +bias)
  v    = x @ W_v^T           [2048, 256]   (natural layout, +bias, +ones col,
                                            emitted in per-head-pair halves)
  s^T[kj,qi] = k_h^T q_h     per head, per 512-wide qi chunk (K=64)
  e^T = exp(s^T / 8)  bf16   (no max subtraction: |s/8| <~ 2, safe)
  ctx[qi,d]  = sum_kj e^T[kj,qi]^T v[kj,d]   NATURAL layout: lhsT = e^T block
               [kj128, qi128], rhs = v [kj128, 65] -> 65-wide moving operand =
               2x fewer PE cycles than the transposed form; col 64 = denom.
               One open PSUM accumulation group per head at a time (PSUM
               zero-regions are bank-wide), lagging the exp production by one
               (pair, qi-chunk) phase.
  normalize on DVE with per-partition scalar (1/denom), write [qi,128] tiles
               pairing the two heads of the group -> cols 0:64 | 64:128
  transpose  ctx [qi,128] -> ctx^T [d,qi] via DMA XBAR (off the PE)
  out^T = W_p_cols @ ctx^T   [1024, 2048]  row-parallel partial projection,
               bf16 partials DMA'd out; host sums 4 partials + b_proj.
"""

import sys

sys.path.insert(0, "/opt/trn_rl_repo")

import ml_dtypes
import numpy as np

import concourse.bass as bass
import concourse.tile as tile
from concourse import bacc, mybir
from concourse.bass_utils import run_bass_kernel_spmd

B, S, D = 2, 2048, 1024
H, HD = 16, 64
HL = 4  # heads per core
P = 128
KC = D // P  # 8 contraction chunks over D
NQ = 4  # qi chunks of 512
NKJ = 16  # kj chunks of 128
F32 = mybir.dt.float32
BF16 = mybir.dt.bfloat16
F8 = mybir.dt.float8e4


def build_program():
    nc = bacc.Bacc("TRN2", target_bir_lowering=False)

    xt_d = nc.dram_tensor("xt", [D, S], BF16, kind="ExternalInput")
    wqk_d = nc.dram_tensor("wqk", [D, 2 * HL * HD], BF16, kind="ExternalInput")
    bqk_d = nc.dram_tensor("bqk", [2 * HL * HD], F32, kind="ExternalInput")
    wv_d = nc.dram_tensor("wv", [D, HL * HD], BF16, kind="ExternalInput")
    bv_d = nc.dram_tensor("bv", [HL * HD], F32, kind="ExternalInput")
    wp_d = nc.dram_tensor("wp", [HL * HD, D], BF16, kind="ExternalInput")
    out_d = nc.dram_tensor("out", [D, S], BF16, kind="ExternalOutput")

    out_v = out_d.rearrange("(mo p) s -> p mo s", p=P)  # [128, 8, 2048]

    with tile.TileContext(nc) as tc:
        with (
            tc.tile_pool(name="const", bufs=1) as const,
            tc.tile_pool(name="xp", bufs=1) as xp,
            tc.tile_pool(name="pexp", bufs=1) as pexp,
            tc.tile_pool(name="prc", bufs=1) as prc,
            tc.tile_pool(name="pcn", bufs=1) as pcn,
            tc.tile_pool(name="pout", bufs=1) as pout,
            tc.tile_pool(name="ps_mm", bufs=2, space="PSUM") as ps_mm,
            tc.tile_pool(name="ps_s", bufs=1, space="PSUM") as ps_s,
            tc.tile_pool(name="ps_o", bufs=1, space="PSUM") as ps_o,
        ):
            # dummy exp so the ACT table load happens during the input DMAs,
            # not on the first real exp
            dum = const.tile([1, 2], F32)
            nc.vector.memset(dum[:], 0.0)
            nc.scalar.activation(dum[:], dum[:], mybir.ActivationFunctionType.Exp)

            # ---- input DMAs, ordered by first use: k-half of wqk, x n0,
            # q-half, x n1, v weights, rest ----
            wqk_v = wqk_d.rearrange("(kc p) m -> p kc m", p=P)
            wqk_k = const.tile([P, KC, 256], BF16, tag="wqk_k")
            nc.sync.dma_start(wqk_k[:], wqk_v[:, :, 256:512])
            bqk_sb = const.tile([P, 4], F32)
            nc.sync.dma_start(bqk_sb[:], bqk_d.rearrange("(m p) -> p m", p=P))
            wv_sb = const.tile([P, KC, 256], BF16)
            bvb_sb = const.tile([P, 256], F32)
            xt_sb = [
                xp.tile([P, KC, 512], BF16, tag=f"xt{n}", name=f"xt{n}")
                for n in range(NQ)
            ]
            xt_v = xt_d.rearrange("(kc p) s -> p kc s", p=P)
            nc.sync.dma_start(xt_sb[0][:], xt_v[:, :, 0:512])
            wqk_q = const.tile([P, KC, 256], BF16, tag="wqk_q")
            nc.sync.dma_start(wqk_q[:], wqk_v[:, :, 0:256])
            nc.sync.dma_start(xt_sb[1][:], xt_v[:, :, 512:1024])
            nc.sync.dma_start(wv_sb[:], wv_d.rearrange("(kc p) m -> p kc m", p=P))
            nc.sync.dma_start(bvb_sb[:], bv_d[:].unsqueeze(0).broadcast_to([P, 256]))
            # xt2/xt3/wp are deferred into the fill schedule so the early q/k
            # remap DMAs aren't queued behind them on the DMA engines
            wp_sb = const.tile([P, 2, D], BF16)

            # ---- qk projection: qk^T [512, 2048], m-chunks 0,1 = q / 2,3 = k.
            # q/k live as fp8e4m3 in two layouts: the natural [128, 512]
            # bias-add output, and the DoubleRow remap [32, (head, khalf),
            # 512] produced by partition-moving DMAs ----
            qk_sb = [
                [
                    const.tile([P, 512], BF16, tag=f"qk{m}n{n}", name=f"qk{m}n{n}")
                    for n in range(NQ)
                ]
                for m in range(4)
            ]
            v_sb = [
                const.tile([P, HL * 65], BF16, tag=f"v{s}", name=f"v{s}")
                for s in range(NKJ)
            ]

            def emit_qk_chunk(m, n):
                pst = ps_mm.tile([P, 512], F32, tag="mm", name="pst")
                w = wqk_q if m < 2 else wqk_k
                mc = m % 2
                for kc in range(KC):
                    nc.tensor.matmul(
                        pst[:],
                        w[:, kc, mc * P : (mc + 1) * P],
                        xt_sb[n][:, kc, :],
                        start=(kc == 0),
                        stop=(kc == KC - 1),
                    )
                nc.vector.tensor_scalar_add(
                    qk_sb[m][n][:], pst[:], bqk_sb[:, m : m + 1]
                )

            def emit_v_chunk(s, half):
                # one head-pair (128 wide) of the v projection for kj chunk s
                pst = ps_mm.tile([P, 256], F32, tag="mm", name="pst")
                nsl = slice(half * P, (half + 1) * P)
                for kc in range(KC):
                    nc.tensor.matmul(
                        pst[:, 0:P],
                        xt_sb[s // 4][:, kc, (s % 4) * P : (s % 4 + 1) * P],
                        wv_sb[:, kc, nsl],
                        start=(kc == 0),
                        stop=(kc == KC - 1),
                    )
                vslice = v_sb[s][:].rearrange("p (h c) -> p h c", h=HL)[
                    :, 2 * half : 2 * half + 2, :
                ]
                psl = pst[:, 0:P].rearrange("p (h c) -> p h c", h=2)
                bsl = bvb_sb[:, nsl].rearrange("p (h c) -> p h c", h=2)
                nc.vector.tensor_add(vslice[:, :, 0:64], psl, bsl)
                # ones column (written as in*0+1 so it exists in bf16)
                nc.vector.tensor_scalar(
                    vslice[:, :, 64:65],
                    psl[:, :, 0:1],
                    0.0,
                    1.0,
                    mybir.AluOpType.mult,
                    mybir.AluOpType.add,
                )

            # ---- attention: 8 phases (pair pr, qi-chunk qc), pr alternating.
            # Scores+exp of phase p run while the AV of phase p-1 contracts
            # its 16 kj chunks one (head, qi-sub-block) accumulation group at
            # a time. ----
            ctxT = [
                [
                    [
                        const.tile(
                            [P, P], BF16, tag=f"ct{qc}p{c}s{sb}", name=f"ct{qc}p{c}s{sb}"
                        )
                        for sb in range(4)
                    ]
                    for c in range(2)
                ]
                for qc in range(NQ)
            ]

            ex_store = {}  # (pr, qc) -> list of (exA, exB) per g
            cn_cur = {}  # sb -> cn tile of the phase being reduced

            def emit_scores_exp(pr, qc, g):
                # fp8e4m3 DoubleRow scores: lhsT = k [32, 2, 128], rhs =
                # q [32, 2, 512] -> one instr per (kj chunk, head) at half
                # the bf16 row cost
                q_tile = qk_sb[pr][qc]
                # shared 3-deep ring (6 PSUM banks): lets the PE run ~1.5 g
                # ahead of ACT so exp never waits at phase boundaries
                psA = ps_s.tile([P, 1024], F32, tag="sA", bufs=1, name="psA")
                psB = ps_s.tile([P, 1024], F32, tag="sB", bufs=1, name="psB")
                for j in range(2):
                    kj = g * 2 + j
                    k_ap = qk_sb[2 + pr][kj // 4]
                    ksl = slice((kj % 4) * P, (kj % 4 + 1) * P)
                    nc.tensor.matmul(
                        psA[:, j * 512 : (j + 1) * 512],
                        k_ap[0:64, ksl],
                        q_tile[0:64, :],
                        start=True,
                        stop=True,
                    )
                exA = pexp.tile([P, 1024], BF16, tag="ex", bufs=34, name="exA")
                nc.scalar.activation(
                    exA[:], psA[:], mybir.ActivationFunctionType.Exp, scale=0.125
                )
                for j in range(2):
                    kj = g * 2 + j
                    k_ap = qk_sb[2 + pr][kj // 4]
                    ksl = slice((kj % 4) * P, (kj % 4 + 1) * P)
                    nc.tensor.matmul(
                        psB[:, j * 512 : (j + 1) * 512],
                        k_ap[64:128, ksl],
                        q_tile[64:128, :],
                        start=True,
                        stop=True,
                    )
                exB = pexp.tile([P, 1024], BF16, tag="ex", bufs=34, name="exB")
                nc.scalar.activation(
                    exB[:], psB[:], mybir.ActivationFunctionType.Exp, scale=0.125
                )
                ex_store[(pr, qc)].append((exA, exB))

            def emit_av_group(pr, qc, gidx, tail=False):
                # one (head, qi-sub-block) accumulation group: contract all 16
                # kj chunks of phase (pr, qc), then normalize; after the head-B
                # half of a sub-block, XBAR-transpose the [qi,128] ctx tile
                head, sb = gidx % 2, gidx // 2
                h = 2 * pr + head
                exs = ex_store[(pr, qc)]
                tag = "poA" if head == 0 else "poB"
                po = ps_o.tile([P, 65], F32, tag=tag, bufs=1, name=tag)
                for kj in range(NKJ):
                    ex = exs[kj // 2][head]
                    nc.tensor.matmul(
                        po[:],
                        ex[:, (kj % 2) * 512 + sb * P : (kj % 2) * 512 + (sb + 1) * P],
                        v_sb[kj][:, h * 65 : h * 65 + 65],
                        start=(kj == 0),
                        stop=(kj == NKJ - 1),
                    )
                rc = prc.tile([P, 1], F32, tag="rc", bufs=4, name="rc")
                nc.vector.reciprocal(rc[:], po[:, 64:65])
                if head == 0:
                    cn_cur[sb] = pcn.tile([P, P], BF16, tag="cn", bufs=4, name="cn")
                cn = cn_cur[sb]
                nc.vector.tensor_scalar_mul(
                    cn[:, head * 64 : head * 64 + 64], po[:, 0:64], rc[:]
                )
                if head == 1:
                    # tail: issue from the (then idle) ACT sequencer; the SP
                    # sequencer is backed up with out-DMAs there
                    eng = nc.scalar if tail else nc.sync
                    eng.dma_start_transpose(ctxT[qc][pr][sb][:], cn[:])

            def emit_proj_mo(qc, mo, tail=False):
                pp = ps_mm.tile([P, 512], F32, tag="mm", name="pp")
                for sb in range(4):
                    for kc2 in range(2):
                        nc.tensor.matmul(
                            pp[:, sb * P : (sb + 1) * P],
                            wp_sb[:, kc2, mo * P : (mo + 1) * P],
                            ctxT[qc][kc2][sb][:],
                            start=(kc2 == 0),
                            stop=(kc2 == 1),
                        )
                ot = pout.tile([P, 512], BF16, tag="ot", bufs=8, name="ot")
                if tail and mo % 2 == 0:
                    # after the last exp the ACT engine is idle: split the
                    # final copy chain across ACT and DVE; out-DMAs stay on
                    # the SP sequencer so they don't serialize behind the
                    # ACT-issued transposes
                    nc.scalar.copy(ot[:], pp[:])
                else:
                    nc.vector.tensor_copy(ot[:], pp[:])
                nc.sync.dma_start(out_v[:, mo, qc * 512 : (qc + 1) * 512], ot[:])

            # Deferred bulk loads: a tiny copy into the destination tile that
            # reads an early-pipeline tile creates a WAW dependency, pinning
            # the DMA behind the q/k remaps in the queue (the scheduler hoists
            # dependency-free DMAs to t=0 otherwise)
            def dma_xt(n, dep):
                nc.vector.tensor_copy(
                    xt_sb[n][0:1, 0:1, 0:2], dep[0:1, 0:2].unsqueeze(1)
                )
                nc.sync.dma_start(xt_sb[n][:], xt_v[:, :, n * 512 : (n + 1) * 512])

            def dma_wp(dep):
                nc.vector.tensor_copy(
                    wp_sb[0:1, 0:1, 0:2], dep[0:1, 0:2].unsqueeze(1)
                )
                nc.sync.dma_start(wp_sb[:], wp_d.rearrange("(kc p) m -> p kc m", p=P))

            # fill work (qkv chunks) per (phase, g), emitted after that g's
            # scores so ACT never waits behind fills
            fills = {
                (0, 0): [lambda: dma_xt(2, qk_sb[2][0]), lambda: emit_qk_chunk(2, 1),
                         lambda: dma_xt(3, qk_sb[2][1])],
                (0, 1): [lambda: emit_v_chunk(0, 0), lambda: emit_v_chunk(1, 0),
                         lambda: emit_v_chunk(2, 0), lambda: emit_v_chunk(3, 0)],
                (0, 2): [lambda: emit_qk_chunk(2, 2), lambda: emit_v_chunk(4, 0),
                         lambda: emit_v_chunk(5, 0)],
                (0, 3): [lambda: emit_v_chunk(6, 0), lambda: emit_v_chunk(7, 0),
                         lambda: emit_v_chunk(8, 0)],
                (0, 4): [lambda: dma_wp(qk_sb[2][2]), lambda: emit_qk_chunk(2, 3),
                         lambda: emit_v_chunk(9, 0), lambda: emit_v_chunk(10, 0)],
                (0, 5): [lambda: emit_v_chunk(11, 0), lambda: emit_v_chunk(12, 0),
                         lambda: emit_v_chunk(13, 0)],
                (0, 6): [lambda: emit_qk_chunk(3, 0), lambda: emit_v_chunk(14, 0),
                         lambda: emit_v_chunk(15, 0)],
                (0, 7): [lambda: emit_qk_chunk(1, 0)],
                (1, 0): [lambda: emit_v_chunk(0, 1), lambda: emit_v_chunk(1, 1)],
                (1, 1): [lambda: emit_qk_chunk(3, 1), lambda: emit_v_chunk(2, 1),
                         lambda: emit_v_chunk(3, 1)],
                (1, 2): [lambda: emit_v_chunk(4, 1), lambda: emit_v_chunk(5, 1)],
                (1, 3): [lambda: emit_qk_chunk(3, 2), lambda: emit_v_chunk(6, 1),
                         lambda: emit_v_chunk(7, 1)],
                (1, 4): [lambda: emit_v_chunk(8, 1), lambda: emit_v_chunk(9, 1)],
                (1, 5): [lambda: emit_qk_chunk(3, 3), lambda: emit_v_chunk(10, 1),
                         lambda: emit_v_chunk(11, 1)],
                (1, 6): [lambda: emit_qk_chunk(0, 1), lambda: emit_v_chunk(12, 1),
                         lambda: emit_v_chunk(13, 1)],
                (1, 7): [lambda: emit_v_chunk(14, 1), lambda: emit_v_chunk(15, 1)],
                (2, 1): [lambda: emit_qk_chunk(1, 1)],
                (2, 3): [lambda: emit_qk_chunk(0, 2)],
                (3, 3): [lambda: emit_qk_chunk(1, 2)],
                (4, 3): [lambda: emit_qk_chunk(0, 3)],
                (5, 3): [lambda: emit_qk_chunk(1, 3)],
            }

            emit_qk_chunk(2, 0)
            emit_qk_chunk(0, 0)

            phases = [(p % 2, p // 2) for p in range(8)]
            for p, (pr, qc) in enumerate(phases):
                ex_store[(pr, qc)] = []
                for g in range(8):
                    if p <= 1:
                        # during the ramp the scores wait on the q/k remap
                        # DMAs anyway; front-running the fills keeps the PE
                        # busy instead of blocking in-order behind them
                        for fill in fills.get((p, g), []):
                            fill()
                        emit_scores_exp(pr, qc, g)
                    else:
                        emit_scores_exp(pr, qc, g)
                        for fill in fills.get((p, g), []):
                            fill()
                    if p >= 1:
                        emit_av_group(*phases[p - 1], g)
                    if p >= 3 and p % 2 == 1 and g >= 1:
                        # proj(qc) once ctxT[qc] is complete (end of phase
                        # 2qc+2); shifted one g late so mo0 never blocks the
                        # in-order PE on the last ctxT transpose's DMA sem
                        emit_proj_mo(p // 2 - 1, g - 1)
                        if g == 7:
                            emit_proj_mo(p // 2 - 1, 7)
            # tail: head-A groups first (they only need the A exps, which land
            # one exp earlier), then B groups whose transposes gate proj(3)
            for gidx in (0, 2, 4, 6, 1, 3, 5, 7):
                emit_av_group(*phases[7], gidx, tail=True)
            for mo in range(8):
                emit_proj_mo(NQ - 1, mo, tail=True)

    nc.compile()
    return nc


_NC = None


def _get_program():
    global _NC
    if _NC is None:
        _NC = build_program()
    return _NC


def prepare_inputs(x, w_qkv, b_qkv, w_proj):
    """Build the 8 per-core input maps from full inputs."""
    in_maps = []
    for c in range(8):
        b, hg = c // 4, c % 4
        sl = slice(hg * 256, (hg + 1) * 256)
        w_q, w_k, w_v = w_qkv[0:D][sl], w_qkv[D : 2 * D][sl], w_qkv[2 * D :][sl]
        in_maps.append(
            {
                "xt": np.ascontiguousarray(x[b].T).astype(ml_dtypes.bfloat16),
                "wqk": np.ascontiguousarray(np.vstack([w_q, w_k]).T).astype(
                    ml_dtypes.bfloat16
                ),
                "bqk": np.concatenate([b_qkv[0:D][sl], b_qkv[D : 2 * D][sl]]),
                "wv": np.ascontiguousarray(w_v.T).astype(ml_dtypes.bfloat16),
                "bv": np.ascontiguousarray(b_qkv[2 * D :][sl]),
                "wp": np.ascontiguousarray(w_proj[:, sl].T).astype(ml_dtypes.bfloat16),
            }
        )
    return in_maps


def run(in_maps, **kwargs):
    nc = _get_program()
    last_err = None
    for _ in range(3):
        try:
            res = run_bass_kernel_spmd(nc, in_maps, core_ids=list(range(8)), **kwargs)
            # force device->host materialization inside the retry scope: lazy
            # jax outputs can surface transient device errors at first access
            res.results = [
                {k: np.array(v, dtype=np.float32) for k, v in r.items()}
                for r in res.results
            ]
            return res
        except Exception as e:  # transient NRT_EXEC_UNIT_UNRECOVERABLE etc.
            last_err = e
    raise last_err


def assemble(results, b_proj):
    out = np.empty((B, S, D), dtype=np.float32)
    for b in range(B):
        acc = results[4 * b]["out"].copy()
        for hg in range(1, 4):
            acc += results[4 * b + hg]["out"]
        out[b] = acc.T + b_proj
    return out


def kernel(x, w_qkv, b_qkv, w_proj, b_proj):
    x = np.asarray(x, dtype=np.float32)
    w_qkv = np.asarray(w_qkv, dtype=np.float32)
    b_qkv = np.asarray(b_qkv, dtype=np.float32)
    w_proj = np.asarray(w_proj, dtype=np.float32)
    b_proj = np.asarray(b_proj, dtype=np.float32)
    res = run(prepare_inputs(x, w_qkv, b_qkv, w_proj))
    return assemble(res.results, b_proj)
